# revision 24
# baseline (speedup 1.0000x reference)
"""TRN2 Bass kernel for nn_KNN_model (conv stack + pairwise patch distances).

Strategy (8 NeuronCores, SPMD):
  - Convs sharded over H: each core computes a 40-row slab (32 owned + 4 halo
    each side) through all 4 conv+BN+ReLU layers in float32r on PE.
    3x3 conv = 6 matmul streams per tile: 3 K=128 pairs (top+mid tap rows via a
    partition-shifted slab copy) + 3 K=64 singles (bottom tap row).
  - BN via raw sums: per-core (sum y, sum y^2) [C,2] (DVE tensor_reduce + ACT
    Square/accum), AllGather, flat DVE add-tree, scale/shift in [C,1] partition
    layout (gamma/beta pre-transposed on host); applied in one ACT pass
    (relu(scale*y+shift)).
  - A dummy collective issued at t=0 absorbs the one-time CC barrier under the
    input DMAs + conv0.
  - Layer-3 BN sums ride in the patch AllGather (row 16), so only ONE
    collective separates conv3 from the distance phase; BN3 is applied to the
    gathered raw patches by every core.
  - Distances: one f32r K=32 matmul per [128,512] tile computes
    -2 q_i.q_j + sq_j (rows 0:16 = -2q, rows 16:32 = ones x q^2); sq_i is an
    exact fp32 per-partition bias fused into the PSUM drain. Drains split
    between DVE (fused add+relu, then big-block ACT sqrt) and direct ACT
    sqrt-from-PSUM (negatives only occur on the diagonal -> NaN, which the
    host overwrites with 0); output written bf16 (host converts to f32).
"""
import numpy as np
import ml_dtypes
import concourse.bacc as bacc
import concourse.bass as bass
import concourse.tile as tile
from concourse import mybir
from concourse.bass_utils import run_bass_kernel_spmd

F32 = mybir.dt.float32
F32R = mybir.dt.float32r
BF16 = mybir.dt.bfloat16
AF = mybir.ActivationFunctionType
ALU = mybir.AluOpType
AX = mybir.AxisListType

NCORES = 8
WP = 258            # padded row width (256 + 2 pad cols)
ROWS = 40           # ext slab rows per core (32 owned + 4 halo each side)
LEAD = 4            # lead margin so tap offsets never go negative
HROWS = 42          # slab rows + 1 pad row top/bottom
HFREE = LEAD + HROWS * WP + 4   # 10844
YFREE = ROWS * WP   # 10320
EPS = 1e-5
RN = 1.0 / 65536.0  # 1 / (global BN sample count per channel)
COUT = [64, 64, 64, 2]
GROUPS = [list(range(NCORES))]
NDVE = 12           # dist n-tiles drained via DVE (rest: direct ACT sqrt)

_CACHE = {}


def _conv_tiles(s0=0, s1=YFREE):
    out, s = [], s0
    while s < s1:
        L = min(512, s1 - s)
        out.append((s, L))
        s += L
    return out


def build():
    nc = bacc.Bacc(trn_type="TRN2", num_devices=NCORES)
    x0 = nc.dram_tensor("x0", [27, YFREE], BF16, kind="ExternalInput").ap()
    w0T = nc.dram_tensor("w0T", [27, 64], BF16, kind="ExternalInput").ap()
    wp_in, ws_in = {}, {}
    for l in (1, 2, 3):
        co = COUT[l]
        wp_in[l] = nc.dram_tensor(f"wp{l}", [3, 128, co], F32, kind="ExternalInput").ap()
        ws_in[l] = nc.dram_tensor(f"ws{l}", [3, 64, co], F32, kind="ExternalInput").ap()
    gbe = nc.dram_tensor("gbe", [64, 8], F32, kind="ExternalInput").ap()
    mask8 = nc.dram_tensor("mask8", [1, 8 * WP], F32, kind="ExternalInput").ap()
    out = nc.dram_tensor("out", [1024, 8192], BF16, kind="ExternalOutput").ap()

    TILES = {0: _conv_tiles(WP, 39 * WP),
             1: _conv_tiles(2 * WP, 38 * WP),
             2: _conv_tiles(3 * WP, 37 * WP)}

    def R(ap):
        return ap.bitcast(F32R)

    with tile.TileContext(nc) as tc:
      with tc.tile_pool(name="pers", bufs=1) as pers, \
           tc.tile_pool(name="dr", bufs=1, space="DRAM") as dr:
        # ---- dummy kick collective: absorbs CC stream barrier at t=0 ----
        kind = dr.tile([1, 1], F32, tag="kind")
        kout = dr.tile([NCORES, 1], F32, tag="kout")
        kt = pers.tile([1, 1], F32)
        nc.vector.memset(kt, 0.0)
        nc.sync.dma_start(out=kind, in_=kt)
        nc.gpsimd.collective_compute(
            "AllGather", ALU.bypass, replica_groups=GROUPS,
            ins=[kind.opt()], outs=[kout.opt()])

        gbes = pers.tile([64, 8], F32)
        nc.sync.dma_start(out=gbes, in_=gbe)
        epst = pers.tile([64, 1], F32)
        nc.vector.memset(epst, EPS)

        def sums_finish(l, C, G, sbp):
            """G [C, 8*2] gathered (sum, sumsq) per core -> scale/shift."""
            H1 = sbp.tile([C, 8], F32, tag=f"H1{l}")
            nc.vector.tensor_add(H1, G[:, 0:8], G[:, 8:16])
            H2 = sbp.tile([C, 4], F32, tag=f"H2{l}")
            nc.vector.tensor_add(H2, H1[:, 0:4], H1[:, 4:8])
            S = sbp.tile([C, 2], F32, tag=f"S{l}")
            nc.vector.tensor_add(S, H2[:, 0:2], H2[:, 2:4])
            mean = sbp.tile([C, 1], F32, tag=f"mn{l}")
            nc.vector.tensor_scalar_mul(mean, S[:, 0:1], RN)
            ey2 = sbp.tile([C, 1], F32, tag=f"e2{l}")
            nc.vector.tensor_scalar_mul(ey2, S[:, 1:2], RN)
            msq = sbp.tile([C, 1], F32, tag=f"mq{l}")
            nc.vector.tensor_mul(msq, mean, mean)
            var = sbp.tile([C, 1], F32, tag=f"va{l}")
            nc.vector.tensor_sub(var, ey2, msq)
            sd = sbp.tile([C, 1], F32, tag=f"sd{l}")
            nc.scalar.activation(sd, var, AF.Sqrt, bias=epst[0:C])
            rs = sbp.tile([C, 1], F32, tag=f"rs{l}")
            nc.vector.reciprocal(rs, sd)
            scl = sbp.tile([C, 1], F32, tag=f"sc{l}")
            nc.vector.tensor_mul(scl, gbes[0:C, l:l + 1], rs)
            tsh = sbp.tile([C, 1], F32, tag=f"ts{l}")
            nc.vector.tensor_mul(tsh, mean, scl)
            sh = sbp.tile([C, 1], F32, tag=f"sh{l}")
            nc.vector.tensor_sub(sh, gbes[0:C, 4 + l:5 + l], tsh)
            return scl, sh

        scr = None

        def sums_partial(l, C, regions, sbp):
            """ACT copy/square + accum_out over regions -> local sums [C,2]."""
            nr = len(regions)
            pa1 = sbp.tile([C, nr], F32, tag=f"q1{l}")
            pa2 = sbp.tile([C, nr], F32, tag=f"p2{l}")
            for j, ap in enumerate(regions):
                nc.scalar.activation(scr[0:C, 0:ap.free_size()], ap, AF.Copy,
                                     accum_out=pa1[:, j:j + 1])
                nc.scalar.activation(scr[0:C, 0:ap.free_size()], ap, AF.Square,
                                     accum_out=pa2[:, j:j + 1])
            pk = sbp.tile([C, 2], F32, tag=f"pk{l}")
            t22 = sbp.tile([C, 4], F32, tag=f"t2{l}")
            nc.vector.tensor_add(t22[:, 0:2], pa1[:, 0:2], pa1[:, 2:4])
            nc.vector.tensor_add(t22[:, 2:4], pa2[:, 0:2], pa2[:, 2:4])
            nc.vector.tensor_add(pk[:, 0:1], t22[:, 0:1], t22[:, 1:2])
            nc.vector.tensor_add(pk[:, 1:2], t22[:, 2:3], t22[:, 3:4])
            return pk

        def bn_crosscore(l, C, pk, sbp):
            """pk [C,2] local sums -> AllGather -> scale/shift [C,1]."""
            sti = dr.tile([C, 2], F32, tag=f"sti{l}")
            sto = dr.tile([NCORES, C, 2], F32, tag=f"sto{l}")
            nc.gpsimd.dma_start(out=sti, in_=pk)
            nc.gpsimd.collective_compute(
                "AllGather", ALU.bypass, replica_groups=GROUPS,
                ins=[sti.opt()], outs=[sto.opt()])
            G = sbp.tile([C, 16], F32, tag=f"G{l}")
            nc.sync.dma_start(out=G.rearrange("c (k t) -> c k t", t=2),
                              in_=sto.rearrange("k c t -> c k t"))
            return sums_finish(l, C, G, sbp)

        # ---------------- conv phase ----------------
        with tc.tile_pool(name="cb", bufs=1) as cb, \
             tc.tile_pool(name="hp", bufs=2) as hp, \
             tc.tile_pool(name="cps", bufs=6, space="PSUM") as cps:
            scr = cb.tile([64, 2560], F32)  # scratch for ACT Square
            x0t = cb.tile([27, YFREE], BF16)
            nc.sync.dma_start(out=x0t[:, 0:5160], in_=x0[:, 0:5160])
            nc.sync.dma_start(out=x0t[:, 5160:YFREE], in_=x0[:, 5160:YFREE])
            mskf = cb.tile([64, 8 * WP], F32)
            nc.sync.dma_start(out=mskf, in_=mask8.partition_broadcast(64))
            mv_ = mskf.rearrange("p (r c) -> p r c", c=WP)
            w0 = cb.tile([27, 64], BF16)
            nc.sync.dma_start(out=w0, in_=w0T)
            wpair, wsing = {}, {}
            for l in (1, 2, 3):
                co = COUT[l]
                for p in range(3):
                    t = cb.tile([128, co], F32R, tag=f"twp{l}{p}")
                    nc.gpsimd.dma_start(out=t, in_=wp_in[l][p])
                    wpair[(l, p)] = t
                    t2 = cb.tile([64, co], F32R, tag=f"tws{l}{p}")
                    nc.gpsimd.dma_start(out=t2, in_=ws_in[l][p])
                    wsing[(l, p)] = t2

            MID = 20 * WP

            def finish_layer(l, yA, yB):
                """BN + ReLU + mask + build padded f32r slab + shifted copy."""
                yAv = yA.rearrange("p (r c) -> p r c", c=WP)
                yBv = yB.rearrange("p (r c) -> p r c", c=WP)
                regions = [yAv[:, 4:12, 1:257], yAv[:, 12:20, 1:257],
                           yBv[:, 0:8, 1:257], yBv[:, 8:16, 1:257]]
                pk = sums_partial(l, 64, regions, cb)
                scl, sh = bn_crosscore(l, 64, pk, cb)
                h = hp.tile([128, HFREE], F32R, tag="h")
                # pad zeroing is independent of scl/sh -> overlaps collective
                nc.vector.memset(h[0:64, 0:LEAD + WP].bitcast(F32), 0.0)
                nc.vector.memset(
                    h[0:64, LEAD + WP + YFREE:HFREE].bitcast(F32), 0.0)
                hv = h[0:64, LEAD + WP:LEAD + WP + YFREE].rearrange(
                    "p (r c) -> p r c", c=WP)
                hcv = h[0:64, LEAD + WP:LEAD + WP + YFREE].rearrange(
                    "p (r c) -> p c r", c=WP)
                # chunk A: y rows 0:20
                nc.scalar.activation(h[0:64, LEAD + WP:LEAD + WP + MID],
                                     yA, AF.Relu, bias=sh, scale=scl)
                nc.vector.tensor_mul(hv[:, 0:4, :], hv[:, 0:4, :], mv_[:, 0:4, :])
                nc.vector.memset(hcv[:, 0, 0:20].bitcast(F32), 0.0)
                nc.vector.memset(hcv[:, 257, 0:20].bitcast(F32), 0.0)
                nc.vector.tensor_copy(h[64:128, 0:LEAD + MID],
                                      h[0:64, WP:LEAD + WP + MID])
                # chunk B: y rows 20:40
                nc.scalar.activation(h[0:64, LEAD + WP + MID:LEAD + WP + YFREE],
                                     yB, AF.Relu, bias=sh, scale=scl)
                nc.vector.tensor_mul(hv[:, 36:40, :], hv[:, 36:40, :], mv_[:, 4:8, :])
                nc.vector.memset(hcv[:, 0, 20:40].bitcast(F32), 0.0)
                nc.vector.memset(hcv[:, 257, 20:40].bitcast(F32), 0.0)
                nc.vector.tensor_copy(h[64:128, LEAD + MID:HFREE - WP],
                                      h[0:64, LEAD + WP + MID:HFREE])
                nc.vector.memset(h[64:128, HFREE - WP:HFREE].bitcast(F32), 0.0)
                return h

            def drain(i, yA, yB, s, L, ps):
                if s + L <= MID:
                    nc.vector.tensor_copy(yA[:, s:s + L], ps[:, 0:L])
                elif s >= MID:
                    nc.vector.tensor_copy(yB[:, s - MID:s - MID + L], ps[:, 0:L])
                else:
                    nc.vector.tensor_copy(yA[:, s:MID], ps[:, 0:MID - s])
                    nc.scalar.copy(yB[:, 0:s + L - MID], ps[:, MID - s:L])

            # conv0 (im2col input, K=27, one stream)
            yA = cb.tile([64, MID], F32, tag="yA")
            yB = cb.tile([64, YFREE - MID], F32, tag="yB")
            for i, (s, L) in enumerate(TILES[0]):
                ps = cps.tile([64, 512], F32, tag="cps")
                nc.tensor.matmul(ps[:, 0:L], w0, x0t[:, s:s + L],
                                 start=True, stop=True)
                drain(i, yA, yB, s, L, ps)
            h = finish_layer(0, yA, yB)

            # conv1, conv2 (6 streams: 3 pairs K=128 + 3 singles K=64)
            GROUP = 6
            for l in (1, 2):
                yA = cb.tile([64, MID], F32, tag="yA")
                yB = cb.tile([64, YFREE - MID], F32, tag="yB")
                for g0 in range(0, len(TILES[l]), GROUP):
                    grp = TILES[l][g0:g0 + GROUP]
                    pss = [cps.tile([64, 512], F32, tag="cps", name=f"cps{g0}_{i}")
                           for i in range(len(grp))]
                    for p in range(3):
                        for ps, (s, L) in zip(pss, grp):
                            o = LEAD + 516 + s + p - 1
                            nc.tensor.matmul(ps[:, 0:L], wsing[(l, p)],
                                             h[0:64, o:o + L],
                                             start=(p == 0), stop=False)
                    for p in range(3):
                        for ps, (s, L) in zip(pss, grp):
                            o = LEAD + s + p - 1
                            nc.tensor.matmul(ps[:, 0:L], wpair[(l, p)],
                                             h[0:128, o:o + L],
                                             start=False, stop=(p == 2))
                    for i, (ps, (s, L)) in enumerate(zip(pss, grp)):
                        drain(g0 + i, yA, yB, s, L, ps)
                h = finish_layer(l, yA, yB)

            # conv3: output streamed in patch order (gy, ph, py, px, gx)
            def c3rhs(p0, np_, off):
                wide = h[p0:p0 + np_, off:off + 2 * WP]
                w2 = wide.rearrange("p (py c) -> p py c", py=2)
                w3 = w2[:, :, 0:256]
                return w3.rearrange("p py (gx px) -> p py px gx", px=4)

            y3 = cb.tile([2, 8192], F32, tag="y3")
            pa1_3 = cb.tile([2, 16], F32)
            pa2_3 = cb.tile([2, 16], F32)
            T3 = [(gy, ph) for gy in range(8) for ph in range(2)]
            for g0 in range(0, 16, 6):
                grp = T3[g0:g0 + 6]
                pss = [cps.tile([64, 512], F32, tag="cps", name=f"cps{g0}_{i}")
                       for i in range(len(grp))]
                bases = [LEAD + (5 + 4 * gy + 2 * ph) * WP + 1 for gy, ph in grp]
                for p in range(3):
                    for ps, base in zip(pss, bases):
                        nc.tensor.matmul(ps[0:2, :], wsing[(3, p)],
                                         c3rhs(0, 64, base + WP + (p - 1)),
                                         start=(p == 0), stop=False)
                for p in range(3):
                    for ps, base in zip(pss, bases):
                        nc.tensor.matmul(ps[0:2, :], wpair[(3, p)],
                                         c3rhs(0, 128, base + (p - 1) - WP),
                                         start=False, stop=(p == 2))
                for i, (ps, (gy, ph)) in enumerate(zip(pss, grp)):
                    t = gy * 2 + ph
                    nc.vector.tensor_scalar(
                        out=y3[:, t * 512:(t + 1) * 512], in0=ps[0:2, :],
                        scalar1=0.0, scalar2=0.0, op0=ALU.add, op1=ALU.add,
                        accum_out=pa1_3[:, t:t + 1])
                    nc.scalar.activation(scr[0:2, 0:512], ps[0:2, :],
                                         AF.Square,
                                         accum_out=pa2_3[:, t:t + 1])

            # local BN3 sums accumulated during the drains
            t8 = cb.tile([2, 16], F32)
            nc.vector.tensor_add(t8[:, 0:8], pa1_3[:, 0:8], pa1_3[:, 8:16])
            nc.vector.tensor_add(t8[:, 8:16], pa2_3[:, 0:8], pa2_3[:, 8:16])
            t4 = cb.tile([2, 8], F32)
            nc.vector.tensor_add(t4[:, 0:4], t8[:, 0:4], t8[:, 4:8])
            nc.vector.tensor_add(t4[:, 4:8], t8[:, 8:12], t8[:, 12:16])
            t2 = cb.tile([2, 4], F32)
            nc.vector.tensor_add(t2[:, 0:2], t4[:, 0:2], t4[:, 2:4])
            nc.vector.tensor_add(t2[:, 2:4], t4[:, 4:6], t4[:, 6:8])
            pk3 = cb.tile([2, 2], F32)
            nc.vector.tensor_add(pk3[:, 0:1], t2[:, 0:1], t2[:, 1:2])
            nc.vector.tensor_add(pk3[:, 1:2], t2[:, 2:3], t2[:, 3:4])

            # scatter raw y3 -> agin patch-major [16,1024] + sums in row 16
            agin = dr.tile([17, 1024], F32, tag="agin")
            gath = dr.tile([NCORES, 17, 1024], F32, tag="gath")
            y5 = y3.rearrange("c (gy py px gx) -> c gy py px gx",
                              gy=8, py=4, px=4)
            agin_r = agin[0:16, :].rearrange(
                "(py px) (c gy gx) -> (py px) c gy gx", py=4, c=2, gy=8)
            for py in range(4):
                for px in range(4):
                    eng = nc.gpsimd if (py * 4 + px) % 2 else nc.sync
                    eng.dma_start(out=agin_r[py * 4 + px],
                                  in_=y5[:, :, py, px, :])
            nc.sync.dma_start(out=agin[16:17, 0:4], in_=pk3)
            nc.gpsimd.collective_compute(
                "AllGather", ALU.bypass, replica_groups=GROUPS,
                ins=[agin.opt()], outs=[gath.opt()])

        # ---------------- distance phase ----------------
        with tc.tile_pool(name="dist", bufs=1) as dist, \
             tc.tile_pool(name="stg", bufs=2) as stg, \
             tc.tile_pool(name="stg2", bufs=2) as stg2, \
             tc.tile_pool(name="dps", bufs=8, space="PSUM") as dps:
            # own raw patches: available pre-gather from agin
            pown = dist.tile([16, 1024], F32)
            nc.sync.dma_start(out=pown, in_=agin[0:16, :])

            # BN3 global stats from gathered row 16
            G3 = dist.tile([2, 16], F32)
            nc.sync.dma_start(
                out=G3.rearrange("c (k t) -> c k t", t=2),
                in_=gath[:, 16, 0:4].rearrange("k (c t) -> c k t", c=2))
            scl3, sh3 = sums_finish(3, 2, G3, dist)
            # broadcast per-channel scale/shift to 16 patch-row partitions
            # (via DRAM so partition_broadcast reads a linear source)
            ssh = dist.tile([2, 2], F32)
            nc.vector.tensor_copy(ssh[:, 0:1], scl3)
            nc.vector.tensor_copy(ssh[:, 1:2], sh3)
            sshd = dr.tile([1, 4], F32, tag="sshd")
            nc.sync.dma_start(out=sshd, in_=ssh)
            sclb, shb = [], []
            for c in range(2):
                sb = dist.tile([16, 1], F32, tag=f"sclb{c}")
                nc.sync.dma_start(
                    out=sb, in_=sshd[:, 2 * c:2 * c + 1].partition_broadcast(16))
                sclb.append(sb)
                hb = dist.tile([16, 1], F32, tag=f"shb{c}")
                nc.sync.dma_start(
                    out=hb, in_=sshd[:, 2 * c + 1:2 * c + 2].partition_broadcast(16))
                shb.append(hb)

            # own q / q^2 / lhsT / sq bias (small, off critical path)
            qown = dist.tile([16, 1024], F32)
            for c in range(2):
                nc.scalar.activation(qown[:, c * 512:(c + 1) * 512],
                                     pown[:, c * 512:(c + 1) * 512],
                                     AF.Relu, bias=shb[c], scale=sclb[c])
            q2own = dist.tile([16, 1024], F32)
            nc.vector.tensor_mul(q2own, qown, qown)
            lhsT2 = dist.tile([32, 1024], F32R)
            nc.vector.tensor_scalar_mul(lhsT2[0:16, :], qown, -2.0)
            onest = dist.tile([16, 1024], F32R)
            nc.vector.memset(onest.bitcast(F32), 1.0)
            nc.sync.dma_start(out=lhsT2[16:32, :], in_=onest)
            ones16 = dist.tile([16, 1], F32)
            nc.vector.memset(ones16, 1.0)
            sqv = dist.tile([1, 1024], F32)
            for j in range(2):
                pq = dps.tile([1, 512], F32, tag="dp", name=f"pq{j}")
                nc.tensor.matmul(pq, ones16, q2own[:, j * 512:(j + 1) * 512],
                                 start=True, stop=True)
                nc.vector.tensor_copy(sqv[:, j * 512:(j + 1) * 512], pq)
            sqd = dr.tile([1, 1024], F32, tag="sqd")
            nc.sync.dma_start(out=sqd, in_=sqv)
            sqT = dist.tile([128, 8], F32)
            nc.sync.dma_start(
                out=sqT, in_=sqd.rearrange("o (m p) -> o p m", p=128))

            # gathered patches -> rhs2 rows 0:16 (q) and 16:32 (q^2)
            praw = dist.tile([16, 8192], F32)
            for c in range(2):
                nc.sync.dma_start(
                    out=praw[:, c * 4096:(c + 1) * 4096].rearrange(
                        "p (k n) -> p k n", k=8),
                    in_=gath[:, 0:16, c * 512:(c + 1) * 512].rearrange(
                        "k p n -> p k n"))
            rhs2 = dist.tile([32, 8192], F32R)
            qsqs = [dist.tile([16, 2048], F32R, tag=f"qsq{i}", name=f"qsq{i}")
                    for i in range(2)]
            for c in range(2):
                for j in range(2):
                    cc = slice(c * 4096 + j * 2048, c * 4096 + (j + 1) * 2048)
                    qsq = qsqs[(c * 2 + j) % 2]
                    nc.scalar.activation(rhs2[0:16, cc], praw[:, cc],
                                         AF.Relu, bias=shb[c], scale=sclb[c])
                    nc.vector.tensor_mul(qsq, rhs2[0:16, cc], rhs2[0:16, cc])
                    nc.sync.dma_start(out=rhs2[16:32, cc], in_=qsq)

            # distance tiles: 8 m-tiles x 16 n-tiles
            for m in range(8):
                stF = stg.tile([128, NDVE * 512], F32, tag="stF")
                st2 = stg2.tile([128, 8192], BF16, tag="st2")
                for n in range(16):
                    ps = dps.tile([128, 512], F32, tag="dp")
                    nc.tensor.matmul(ps, lhsT2[0:32, m * 128:(m + 1) * 128],
                                     rhs2[0:32, n * 512:(n + 1) * 512],
                                     start=True, stop=True)
                    if n < NDVE:
                        nc.vector.tensor_scalar(
                            out=stF[:, n * 512:(n + 1) * 512], in0=ps,
                            scalar1=sqT[:, m:m + 1], scalar2=0.0,
                            op0=ALU.add, op1=ALU.max)
                        if n == NDVE // 2 - 1:
                            nc.scalar.activation(
                                st2[:, 0:(NDVE // 2) * 512],
                                stF[:, 0:(NDVE // 2) * 512], AF.Sqrt)
                        elif n == NDVE - 1:
                            nc.scalar.activation(
                                st2[:, (NDVE // 2) * 512:NDVE * 512],
                                stF[:, (NDVE // 2) * 512:NDVE * 512], AF.Sqrt)
                    else:
                        nc.scalar.activation(
                            st2[:, n * 512:(n + 1) * 512], ps, AF.Sqrt,
                            bias=sqT[:, m:m + 1])
                nc.sync.dma_start(out=out[m * 128:(m + 1) * 128, :], in_=st2)
    nc.finalize()
    return nc


def _prep_inputs(x, ws_, gs, bes):
    """Per-core numpy input dicts."""
    xp = np.pad(x[0], ((0, 0), (5, 5), (2, 3))).astype(np.float32)
    w0 = ws_[0]
    w0T = np.ascontiguousarray(
        w0.transpose(2, 3, 1, 0).reshape(27, 64)).astype(np.float32)
    wp, wsg = {}, {}
    for l in (1, 2, 3):
        w = ws_[l]
        wp[l] = np.ascontiguousarray(np.stack(
            [np.concatenate([w[:, :, 0, p].T, w[:, :, 1, p].T], 0)
             for p in range(3)])).astype(np.float32)
        wsg[l] = np.ascontiguousarray(np.stack(
            [w[:, :, 2, p].T for p in range(3)])).astype(np.float32)
    gbe = np.zeros((64, 8), np.float32)
    for l in range(4):
        g = np.asarray(gs[l], np.float32).ravel()
        b = np.asarray(bes[l], np.float32).ravel()
        gbe[:len(g), l] = g
        gbe[:len(b), 4 + l] = b
    in_maps = []
    for k in range(NCORES):
        col = np.empty((27, ROWS, WP), np.float32)
        for dy in range(3):
            for dx in range(3):
                for ci in range(3):
                    r0 = 32 * k + dy
                    col[(dy * 3 + dx) * 3 + ci] = xp[ci, r0:r0 + ROWS, dx:dx + WP]
        mask = np.zeros((8, WP), np.float32)
        for i, r in enumerate([0, 1, 2, 3, 36, 37, 38, 39]):
            ir = 32 * k - 4 + r
            if 0 <= ir < 256:
                mask[i, 1:257] = 1.0
        in_maps.append(dict(
            x0=np.ascontiguousarray(col.reshape(27, YFREE)).astype(
                ml_dtypes.bfloat16),
            w0T=w0T.astype(ml_dtypes.bfloat16), wp1=wp[1], ws1=wsg[1], wp2=wp[2], ws2=wsg[2],
            wp3=wp[3], ws3=wsg[3], gbe=gbe,
            mask8=np.ascontiguousarray(mask.reshape(1, 8 * WP))))
    return in_maps


def kernel(x, w0, b0, g0, be0, w1, b1, g1, be1, w2, b2, g2, be2,
           w3, b3, g3, be3):
    # conv bias b_i cancels exactly inside BatchNorm (mean absorbs it); unused.
    if "nc" not in _CACHE:
        _CACHE["nc"] = build()
    nc = _CACHE["nc"]
    in_maps = _prep_inputs(
        np.asarray(x, np.float32),
        [np.asarray(w, np.float32) for w in (w0, w1, w2, w3)],
        (g0, g1, g2, g3), (be0, be1, be2, be3))
    res = run_bass_kernel_spmd(nc, in_maps, list(range(NCORES)))
    D = np.empty((8192, 8192), np.float32)
    for k in range(NCORES):
        o = np.asarray(res.results[k]["out"]).astype(np.float32)
        for c in range(2):
            D[c * 4096 + k * 512: c * 4096 + (k + 1) * 512, :] = \
                o[c * 512:(c + 1) * 512, :]
    np.fill_diagonal(D, 0.0)
    return D


# revision 26
# speedup vs baseline: 1.1821x; 1.1821x over previous
"""TRN2 Bass kernel for nn_KNN_model (conv stack + pairwise patch distances).

Strategy (8 NeuronCores, SPMD):
  - Convs sharded over H: each core computes a 40-row slab (32 owned + 4 halo
    each side) through all 4 conv+BN+ReLU layers in float32r on PE.
    3x3 conv = 6 matmul streams per tile: 3 K=128 pairs (top+mid tap rows via a
    partition-shifted slab copy) + 3 K=64 singles (bottom tap row).
  - BN via raw sums: per-core (sum y, sum y^2) [C,2] (DVE tensor_reduce + ACT
    Square/accum), AllGather, flat DVE add-tree, scale/shift in [C,1] partition
    layout (gamma/beta pre-transposed on host); applied in one ACT pass
    (relu(scale*y+shift)).
  - A dummy collective issued at t=0 absorbs the one-time CC barrier under the
    input DMAs + conv0.
  - Layer-3 BN sums ride in the patch AllGather (row 16), so only ONE
    collective separates conv3 from the distance phase; BN3 is applied to the
    gathered raw patches by every core.
  - Distances: one f32r K=32 matmul per [128,512] tile computes
    -2 q_i.q_j + sq_j (rows 0:16 = -2q, rows 16:32 = ones x q^2); sq_i is an
    exact fp32 per-partition bias fused into the PSUM drain. Drains split
    between DVE (fused add+relu, then big-block ACT sqrt) and direct ACT
    sqrt-from-PSUM (negatives only occur on the diagonal -> NaN, which the
    host overwrites with 0); output written bf16 (host converts to f32).
"""
import numpy as np
import ml_dtypes
import concourse.bacc as bacc
import concourse.bass as bass
import concourse.tile as tile
from concourse import mybir
from concourse.bass_utils import run_bass_kernel_spmd

F32 = mybir.dt.float32
F32R = mybir.dt.float32r
BF16 = mybir.dt.bfloat16
AF = mybir.ActivationFunctionType
ALU = mybir.AluOpType
AX = mybir.AxisListType

NCORES = 8
WP = 258            # padded row width (256 + 2 pad cols)
ROWS = 40           # ext slab rows per core (32 owned + 4 halo each side)
LEAD = 4            # lead margin so tap offsets never go negative
HROWS = 42          # slab rows + 1 pad row top/bottom
HFREE = LEAD + HROWS * WP + 4   # 10844
YFREE = ROWS * WP   # 10320
EPS = 1e-5
RN = 1.0 / 65536.0  # 1 / (global BN sample count per channel)
COUT = [64, 64, 64, 2]
GROUPS = [list(range(NCORES))]
NDVE = 12           # dist n-tiles drained via DVE (rest: direct ACT sqrt)

_CACHE = {}


def _conv_tiles(s0=0, s1=YFREE):
    out, s = [], s0
    while s < s1:
        L = min(512, s1 - s)
        out.append((s, L))
        s += L
    return out


def build():
    nc = bacc.Bacc(trn_type="TRN2", num_devices=NCORES)
    x0 = nc.dram_tensor("x0", [27, YFREE], BF16, kind="ExternalInput").ap()
    w0T = nc.dram_tensor("w0T", [27, 64], BF16, kind="ExternalInput").ap()
    wp_in, ws_in = {}, {}
    for l in (1, 2, 3):
        co = COUT[l]
        wp_in[l] = nc.dram_tensor(f"wp{l}", [3, 128, co], F32, kind="ExternalInput").ap()
        ws_in[l] = nc.dram_tensor(f"ws{l}", [3, 64, co], F32, kind="ExternalInput").ap()
    gbe = nc.dram_tensor("gbe", [64, 8], F32, kind="ExternalInput").ap()
    mask8 = nc.dram_tensor("mask8", [1, 8 * WP], F32, kind="ExternalInput").ap()
    out = nc.dram_tensor("out", [1024, 8192], BF16, kind="ExternalOutput").ap()

    TILES = {0: _conv_tiles(WP, 39 * WP),
             1: _conv_tiles(2 * WP, 38 * WP),
             2: _conv_tiles(3 * WP, 37 * WP)}

    def R(ap):
        return ap.bitcast(F32R)

    with tile.TileContext(nc) as tc:
      with tc.tile_pool(name="pers", bufs=1) as pers, \
           tc.tile_pool(name="dr", bufs=1, space="DRAM") as dr:
        gbes = pers.tile([64, 8], F32)
        nc.sync.dma_start(out=gbes, in_=gbe)
        epst = pers.tile([64, 1], F32)
        nc.vector.memset(epst, EPS)

        def sums_finish(l, C, G, sbp):
            """G [C, 8*2] gathered (sum, sumsq) per core -> scale/shift."""
            H1 = sbp.tile([C, 8], F32, tag=f"H1{l}")
            nc.vector.tensor_add(H1, G[:, 0:8], G[:, 8:16])
            H2 = sbp.tile([C, 4], F32, tag=f"H2{l}")
            nc.vector.tensor_add(H2, H1[:, 0:4], H1[:, 4:8])
            S = sbp.tile([C, 2], F32, tag=f"S{l}")
            nc.vector.tensor_add(S, H2[:, 0:2], H2[:, 2:4])
            mean = sbp.tile([C, 1], F32, tag=f"mn{l}")
            nc.vector.tensor_scalar_mul(mean, S[:, 0:1], RN)
            ey2 = sbp.tile([C, 1], F32, tag=f"e2{l}")
            nc.vector.tensor_scalar_mul(ey2, S[:, 1:2], RN)
            msq = sbp.tile([C, 1], F32, tag=f"mq{l}")
            nc.vector.tensor_mul(msq, mean, mean)
            var = sbp.tile([C, 1], F32, tag=f"va{l}")
            nc.vector.tensor_sub(var, ey2, msq)
            sd = sbp.tile([C, 1], F32, tag=f"sd{l}")
            nc.scalar.activation(sd, var, AF.Sqrt, bias=epst[0:C])
            rs = sbp.tile([C, 1], F32, tag=f"rs{l}")
            nc.vector.reciprocal(rs, sd)
            scl = sbp.tile([C, 1], F32, tag=f"sc{l}")
            nc.vector.tensor_mul(scl, gbes[0:C, l:l + 1], rs)
            tsh = sbp.tile([C, 1], F32, tag=f"ts{l}")
            nc.vector.tensor_mul(tsh, mean, scl)
            sh = sbp.tile([C, 1], F32, tag=f"sh{l}")
            nc.vector.tensor_sub(sh, gbes[0:C, 4 + l:5 + l], tsh)
            return scl, sh

        scr = None

        def sums_partial(l, C, regions, sbp):
            """ACT copy/square + accum_out over regions -> local sums [C,2]."""
            nr = len(regions)
            pa1 = sbp.tile([C, nr], F32, tag=f"q1{l}")
            pa2 = sbp.tile([C, nr], F32, tag=f"p2{l}")
            for j, ap in enumerate(regions):
                nc.vector.tensor_reduce(out=pa1[:, j:j + 1], in_=ap,
                                        axis=AX.XY, op=ALU.add)
                nc.scalar.activation(scr[0:C, 0:ap.free_size()], ap, AF.Square,
                                     accum_out=pa2[:, j:j + 1])
            pk = sbp.tile([C, 2], F32, tag=f"pk{l}")
            t22 = sbp.tile([C, 4], F32, tag=f"t2{l}")
            nc.vector.tensor_add(t22[:, 0:2], pa1[:, 0:2], pa1[:, 2:4])
            nc.vector.tensor_add(t22[:, 2:4], pa2[:, 0:2], pa2[:, 2:4])
            nc.vector.tensor_add(pk[:, 0:1], t22[:, 0:1], t22[:, 1:2])
            nc.vector.tensor_add(pk[:, 1:2], t22[:, 2:3], t22[:, 3:4])
            return pk

        def bn_crosscore(l, C, pk, sbp):
            """pk [C,2] local sums -> AllGather -> scale/shift [C,1]."""
            sti = dr.tile([C, 2], F32, tag=f"sti{l}")
            sto = dr.tile([NCORES, C, 2], F32, tag=f"sto{l}")
            nc.gpsimd.dma_start(out=sti, in_=pk)
            nc.gpsimd.collective_compute(
                "AllGather", ALU.bypass, replica_groups=GROUPS,
                ins=[sti.opt()], outs=[sto.opt()])
            G = sbp.tile([C, 16], F32, tag=f"G{l}")
            nc.sync.dma_start(out=G.rearrange("c (k t) -> c k t", t=2),
                              in_=sto.rearrange("k c t -> c k t"))
            return sums_finish(l, C, G, sbp)

        # ---------------- conv phase ----------------
        with tc.tile_pool(name="cb", bufs=1) as cb, \
             tc.tile_pool(name="hp", bufs=2) as hp, \
             tc.tile_pool(name="cps", bufs=6, space="PSUM") as cps:
            scr = cb.tile([64, 2560], F32)  # scratch for ACT Square
            x0t = cb.tile([27, YFREE], BF16)
            nc.sync.dma_start(out=x0t[:, 0:5160], in_=x0[:, 0:5160])
            nc.sync.dma_start(out=x0t[:, 5160:YFREE], in_=x0[:, 5160:YFREE])
            mskf = cb.tile([64, 8 * WP], F32)
            nc.sync.dma_start(out=mskf, in_=mask8.partition_broadcast(64))
            mv_ = mskf.rearrange("p (r c) -> p r c", c=WP)
            w0 = cb.tile([27, 64], BF16)
            nc.sync.dma_start(out=w0, in_=w0T)
            wpair, wsing = {}, {}
            for l in (1, 2, 3):
                co = COUT[l]
                for p in range(3):
                    t = cb.tile([128, co], F32R, tag=f"twp{l}{p}")
                    nc.gpsimd.dma_start(out=t, in_=wp_in[l][p])
                    wpair[(l, p)] = t
                    t2 = cb.tile([64, co], F32R, tag=f"tws{l}{p}")
                    nc.gpsimd.dma_start(out=t2, in_=ws_in[l][p])
                    wsing[(l, p)] = t2

            MID = 20 * WP

            def finish_layer(l, yA, yB):
                """BN + ReLU + mask + build padded f32r slab + shifted copy."""
                yAv = yA.rearrange("p (r c) -> p r c", c=WP)
                yBv = yB.rearrange("p (r c) -> p r c", c=WP)
                regions = [yAv[:, 4:12, 1:257], yAv[:, 12:20, 1:257],
                           yBv[:, 0:8, 1:257], yBv[:, 8:16, 1:257]]
                pk = sums_partial(l, 64, regions, cb)
                scl, sh = bn_crosscore(l, 64, pk, cb)
                h = hp.tile([128, HFREE], F32R, tag="h")
                # pad zeroing is independent of scl/sh -> overlaps collective
                nc.vector.memset(h[0:64, 0:LEAD + WP].bitcast(F32), 0.0)
                nc.vector.memset(
                    h[0:64, LEAD + WP + YFREE:HFREE].bitcast(F32), 0.0)
                hv = h[0:64, LEAD + WP:LEAD + WP + YFREE].rearrange(
                    "p (r c) -> p r c", c=WP)
                hcv = h[0:64, LEAD + WP:LEAD + WP + YFREE].rearrange(
                    "p (r c) -> p c r", c=WP)
                # chunk A: y rows 0:20
                nc.scalar.activation(h[0:64, LEAD + WP:LEAD + WP + MID],
                                     yA, AF.Relu, bias=sh, scale=scl)
                nc.vector.tensor_mul(hv[:, 0:4, :], hv[:, 0:4, :], mv_[:, 0:4, :])
                nc.vector.memset(hcv[:, 0, 0:20].bitcast(F32), 0.0)
                nc.vector.memset(hcv[:, 257, 0:20].bitcast(F32), 0.0)
                nc.vector.tensor_copy(h[64:128, 0:LEAD + MID],
                                      h[0:64, WP:LEAD + WP + MID])
                # chunk B: y rows 20:40
                nc.scalar.activation(h[0:64, LEAD + WP + MID:LEAD + WP + YFREE],
                                     yB, AF.Relu, bias=sh, scale=scl)
                nc.vector.tensor_mul(hv[:, 36:40, :], hv[:, 36:40, :], mv_[:, 4:8, :])
                nc.vector.memset(hcv[:, 0, 20:40].bitcast(F32), 0.0)
                nc.vector.memset(hcv[:, 257, 20:40].bitcast(F32), 0.0)
                nc.vector.tensor_copy(h[64:128, LEAD + MID:HFREE - WP],
                                      h[0:64, LEAD + WP + MID:HFREE])
                nc.vector.memset(h[64:128, HFREE - WP:HFREE].bitcast(F32), 0.0)
                return h

            def drain(i, yA, yB, s, L, ps):
                if s + L <= MID:
                    nc.vector.tensor_copy(yA[:, s:s + L], ps[:, 0:L])
                elif s >= MID:
                    nc.vector.tensor_copy(yB[:, s - MID:s - MID + L], ps[:, 0:L])
                else:
                    nc.vector.tensor_copy(yA[:, s:MID], ps[:, 0:MID - s])
                    nc.scalar.copy(yB[:, 0:s + L - MID], ps[:, MID - s:L])

            # conv0 (im2col input, K=27, one stream)
            yA = cb.tile([64, MID], F32, tag="yA")
            yB = cb.tile([64, YFREE - MID], F32, tag="yB")
            for i, (s, L) in enumerate(TILES[0]):
                ps = cps.tile([64, 512], F32, tag="cps")
                nc.tensor.matmul(ps[:, 0:L], w0, x0t[:, s:s + L],
                                 start=True, stop=True)
                drain(i, yA, yB, s, L, ps)
            h = finish_layer(0, yA, yB)

            # conv1, conv2 (6 streams: 3 pairs K=128 + 3 singles K=64)
            GROUP = 6
            for l in (1, 2):
                yA = cb.tile([64, MID], F32, tag="yA")
                yB = cb.tile([64, YFREE - MID], F32, tag="yB")
                for g0 in range(0, len(TILES[l]), GROUP):
                    grp = TILES[l][g0:g0 + GROUP]
                    pss = [cps.tile([64, 512], F32, tag="cps", name=f"cps{g0}_{i}")
                           for i in range(len(grp))]
                    for p in range(3):
                        for ps, (s, L) in zip(pss, grp):
                            o = LEAD + 516 + s + p - 1
                            nc.tensor.matmul(ps[:, 0:L], wsing[(l, p)],
                                             h[0:64, o:o + L],
                                             start=(p == 0), stop=False)
                    for p in range(3):
                        for ps, (s, L) in zip(pss, grp):
                            o = LEAD + s + p - 1
                            nc.tensor.matmul(ps[:, 0:L], wpair[(l, p)],
                                             h[0:128, o:o + L],
                                             start=False, stop=(p == 2))
                    for i, (ps, (s, L)) in enumerate(zip(pss, grp)):
                        drain(g0 + i, yA, yB, s, L, ps)
                h = finish_layer(l, yA, yB)

            # conv3: output streamed in patch order (gy, ph, py, px, gx)
            def c3rhs(p0, np_, off):
                wide = h[p0:p0 + np_, off:off + 2 * WP]
                w2 = wide.rearrange("p (py c) -> p py c", py=2)
                w3 = w2[:, :, 0:256]
                return w3.rearrange("p py (gx px) -> p py px gx", px=4)

            y3 = cb.tile([2, 8192], F32, tag="y3")
            pa1_3 = cb.tile([2, 16], F32)
            pa2_3 = cb.tile([2, 16], F32)
            T3 = [(gy, ph) for gy in range(8) for ph in range(2)]
            for g0 in range(0, 16, 6):
                grp = T3[g0:g0 + 6]
                pss = [cps.tile([64, 512], F32, tag="cps", name=f"cps{g0}_{i}")
                       for i in range(len(grp))]
                bases = [LEAD + (5 + 4 * gy + 2 * ph) * WP + 1 for gy, ph in grp]
                for p in range(3):
                    for ps, base in zip(pss, bases):
                        nc.tensor.matmul(ps[0:2, :], wsing[(3, p)],
                                         c3rhs(0, 64, base + WP + (p - 1)),
                                         start=(p == 0), stop=False)
                for p in range(3):
                    for ps, base in zip(pss, bases):
                        nc.tensor.matmul(ps[0:2, :], wpair[(3, p)],
                                         c3rhs(0, 128, base + (p - 1) - WP),
                                         start=False, stop=(p == 2))
                for i, (ps, (gy, ph)) in enumerate(zip(pss, grp)):
                    t = gy * 2 + ph
                    nc.vector.tensor_scalar(
                        out=y3[:, t * 512:(t + 1) * 512], in0=ps[0:2, :],
                        scalar1=0.0, scalar2=0.0, op0=ALU.add, op1=ALU.add,
                        accum_out=pa1_3[:, t:t + 1])
                    nc.scalar.activation(scr[0:2, 0:512], ps[0:2, :],
                                         AF.Square,
                                         accum_out=pa2_3[:, t:t + 1])

            # local BN3 sums accumulated during the drains
            t8 = cb.tile([2, 16], F32)
            nc.vector.tensor_add(t8[:, 0:8], pa1_3[:, 0:8], pa1_3[:, 8:16])
            nc.vector.tensor_add(t8[:, 8:16], pa2_3[:, 0:8], pa2_3[:, 8:16])
            t4 = cb.tile([2, 8], F32)
            nc.vector.tensor_add(t4[:, 0:4], t8[:, 0:4], t8[:, 4:8])
            nc.vector.tensor_add(t4[:, 4:8], t8[:, 8:12], t8[:, 12:16])
            t2 = cb.tile([2, 4], F32)
            nc.vector.tensor_add(t2[:, 0:2], t4[:, 0:2], t4[:, 2:4])
            nc.vector.tensor_add(t2[:, 2:4], t4[:, 4:6], t4[:, 6:8])
            pk3 = cb.tile([2, 2], F32)
            nc.vector.tensor_add(pk3[:, 0:1], t2[:, 0:1], t2[:, 1:2])
            nc.vector.tensor_add(pk3[:, 1:2], t2[:, 2:3], t2[:, 3:4])

            # scatter raw y3 -> agin patch-major [16,1024] + sums in row 16
            agin = dr.tile([17, 1024], F32, tag="agin")
            gath = dr.tile([NCORES, 17, 1024], F32, tag="gath")
            y5 = y3.rearrange("c (gy py px gx) -> c gy py px gx",
                              gy=8, py=4, px=4)
            agin_r = agin[0:16, :].rearrange(
                "(py px) (c gy gx) -> (py px) c gy gx", py=4, c=2, gy=8)
            for py in range(4):
                for px in range(4):
                    eng = nc.gpsimd if (py * 4 + px) % 2 else nc.sync
                    eng.dma_start(out=agin_r[py * 4 + px],
                                  in_=y5[:, :, py, px, :])
            nc.sync.dma_start(out=agin[16:17, 0:4], in_=pk3)
            nc.gpsimd.collective_compute(
                "AllGather", ALU.bypass, replica_groups=GROUPS,
                ins=[agin.opt()], outs=[gath.opt()])

        # ---------------- distance phase ----------------
        with tc.tile_pool(name="dist", bufs=1) as dist, \
             tc.tile_pool(name="stg", bufs=2) as stg, \
             tc.tile_pool(name="stg2", bufs=2) as stg2, \
             tc.tile_pool(name="dps", bufs=8, space="PSUM") as dps:
            # own raw patches: available pre-gather from agin
            pown = dist.tile([16, 1024], F32)
            nc.sync.dma_start(out=pown, in_=agin[0:16, :])

            # BN3 global stats from gathered row 16
            G3 = dist.tile([2, 16], F32)
            nc.sync.dma_start(
                out=G3.rearrange("c (k t) -> c k t", t=2),
                in_=gath[:, 16, 0:4].rearrange("k (c t) -> c k t", c=2))
            scl3, sh3 = sums_finish(3, 2, G3, dist)
            # broadcast per-channel scale/shift to 16 patch-row partitions
            # (via DRAM so partition_broadcast reads a linear source)
            ssh = dist.tile([2, 2], F32)
            nc.vector.tensor_copy(ssh[:, 0:1], scl3)
            nc.vector.tensor_copy(ssh[:, 1:2], sh3)
            sshd = dr.tile([1, 4], F32, tag="sshd")
            nc.sync.dma_start(out=sshd, in_=ssh)
            sclb, shb = [], []
            for c in range(2):
                sb = dist.tile([16, 1], F32, tag=f"sclb{c}")
                nc.sync.dma_start(
                    out=sb, in_=sshd[:, 2 * c:2 * c + 1].partition_broadcast(16))
                sclb.append(sb)
                hb = dist.tile([16, 1], F32, tag=f"shb{c}")
                nc.sync.dma_start(
                    out=hb, in_=sshd[:, 2 * c + 1:2 * c + 2].partition_broadcast(16))
                shb.append(hb)

            # own q / q^2 / lhsT / sq bias (small, off critical path)
            qown = dist.tile([16, 1024], F32)
            for c in range(2):
                nc.scalar.activation(qown[:, c * 512:(c + 1) * 512],
                                     pown[:, c * 512:(c + 1) * 512],
                                     AF.Relu, bias=shb[c], scale=sclb[c])
            q2own = dist.tile([16, 1024], F32)
            nc.vector.tensor_mul(q2own, qown, qown)
            lhsT2 = dist.tile([96, 1024], F32R)
            nc.vector.tensor_scalar_mul(lhsT2[0:16, :], qown, -2.0)
            onest = dist.tile([16, 1024], F32R)
            nc.vector.memset(onest.bitcast(F32), 1.0)
            nc.sync.dma_start(out=lhsT2[16:32, :], in_=onest)
            nc.sync.dma_start(out=lhsT2[64:96, :], in_=lhsT2[0:32, :])
            ones16 = dist.tile([16, 1], F32)
            nc.vector.memset(ones16, 1.0)
            sqv = dist.tile([1, 1024], F32)
            for j in range(2):
                pq = dps.tile([1, 512], F32, tag="dp", name=f"pq{j}")
                nc.tensor.matmul(pq, ones16, q2own[:, j * 512:(j + 1) * 512],
                                 start=True, stop=True)
                nc.vector.tensor_copy(sqv[:, j * 512:(j + 1) * 512], pq)
            sqd = dr.tile([1, 1024], F32, tag="sqd")
            nc.sync.dma_start(out=sqd, in_=sqv)
            sqT = dist.tile([128, 8], F32)
            nc.sync.dma_start(
                out=sqT, in_=sqd.rearrange("o (m p) -> o p m", p=128))

            # gathered patches -> rhs2 rows 0:16 (q) and 16:32 (q^2)
            praw = dist.tile([16, 8192], F32)
            for c in range(2):
                nc.sync.dma_start(
                    out=praw[:, c * 4096:(c + 1) * 4096].rearrange(
                        "p (k n) -> p k n", k=8),
                    in_=gath[:, 0:16, c * 512:(c + 1) * 512].rearrange(
                        "k p n -> p k n"))
            rhs2 = dist.tile([96, 8192], F32R)
            qsqs = [dist.tile([16, 2048], F32R, tag=f"qsq{i}", name=f"qsq{i}")
                    for i in range(2)]
            for c in range(2):
                for j in range(2):
                    cc = slice(c * 4096 + j * 2048, c * 4096 + (j + 1) * 2048)
                    qsq = qsqs[(c * 2 + j) % 2]
                    nc.scalar.activation(rhs2[0:16, cc], praw[:, cc],
                                         AF.Relu, bias=shb[c], scale=sclb[c])
                    nc.vector.tensor_mul(qsq, rhs2[0:16, cc], rhs2[0:16, cc])
                    nc.sync.dma_start(out=rhs2[16:32, cc], in_=qsq)
                    nc.sync.dma_start(out=rhs2[64:96, cc], in_=rhs2[0:32, cc])

            # distance tiles: 8 m-tiles x 16 n-tiles
            for m in range(8):
                stF = stg.tile([128, NDVE * 512], F32, tag="stF")
                st2 = stg2.tile([128, 8192], BF16, tag="st2")
                for n in range(16):
                    b = 64 * (n % 2)
                    ps = dps.tile([128, 512], F32, tag="dp")
                    nc.tensor.matmul(ps, lhsT2[b:b + 32, m * 128:(m + 1) * 128],
                                     rhs2[b:b + 32, n * 512:(n + 1) * 512],
                                     start=True, stop=True,
                                     tile_position=(b, 0))
                    if n < NDVE:
                        nc.vector.tensor_scalar(
                            out=stF[:, n * 512:(n + 1) * 512], in0=ps,
                            scalar1=sqT[:, m:m + 1], scalar2=0.0,
                            op0=ALU.add, op1=ALU.max)
                        if n == NDVE // 2 - 1:
                            nc.scalar.activation(
                                st2[:, 0:(NDVE // 2) * 512],
                                stF[:, 0:(NDVE // 2) * 512], AF.Sqrt)
                        elif n == NDVE - 1:
                            nc.scalar.activation(
                                st2[:, (NDVE // 2) * 512:NDVE * 512],
                                stF[:, (NDVE // 2) * 512:NDVE * 512], AF.Sqrt)
                    else:
                        nc.scalar.activation(
                            st2[:, n * 512:(n + 1) * 512], ps, AF.Sqrt,
                            bias=sqT[:, m:m + 1])
                nc.sync.dma_start(out=out[m * 128:(m + 1) * 128, :], in_=st2)
    nc.finalize()
    return nc


def _prep_inputs(x, ws_, gs, bes):
    """Per-core numpy input dicts."""
    xp = np.pad(x[0], ((0, 0), (5, 5), (2, 3))).astype(np.float32)
    w0 = ws_[0]
    w0T = np.ascontiguousarray(
        w0.transpose(2, 3, 1, 0).reshape(27, 64)).astype(np.float32)
    wp, wsg = {}, {}
    for l in (1, 2, 3):
        w = ws_[l]
        wp[l] = np.ascontiguousarray(np.stack(
            [np.concatenate([w[:, :, 0, p].T, w[:, :, 1, p].T], 0)
             for p in range(3)])).astype(np.float32)
        wsg[l] = np.ascontiguousarray(np.stack(
            [w[:, :, 2, p].T for p in range(3)])).astype(np.float32)
    gbe = np.zeros((64, 8), np.float32)
    for l in range(4):
        g = np.asarray(gs[l], np.float32).ravel()
        b = np.asarray(bes[l], np.float32).ravel()
        gbe[:len(g), l] = g
        gbe[:len(b), 4 + l] = b
    in_maps = []
    for k in range(NCORES):
        col = np.empty((27, ROWS, WP), np.float32)
        for dy in range(3):
            for dx in range(3):
                for ci in range(3):
                    r0 = 32 * k + dy
                    col[(dy * 3 + dx) * 3 + ci] = xp[ci, r0:r0 + ROWS, dx:dx + WP]
        mask = np.zeros((8, WP), np.float32)
        for i, r in enumerate([0, 1, 2, 3, 36, 37, 38, 39]):
            ir = 32 * k - 4 + r
            if 0 <= ir < 256:
                mask[i, 1:257] = 1.0
        in_maps.append(dict(
            x0=np.ascontiguousarray(col.reshape(27, YFREE)).astype(
                ml_dtypes.bfloat16),
            w0T=w0T.astype(ml_dtypes.bfloat16), wp1=wp[1], ws1=wsg[1], wp2=wp[2], ws2=wsg[2],
            wp3=wp[3], ws3=wsg[3], gbe=gbe,
            mask8=np.ascontiguousarray(mask.reshape(1, 8 * WP))))
    return in_maps


def kernel(x, w0, b0, g0, be0, w1, b1, g1, be1, w2, b2, g2, be2,
           w3, b3, g3, be3):
    # conv bias b_i cancels exactly inside BatchNorm (mean absorbs it); unused.
    if "nc" not in _CACHE:
        _CACHE["nc"] = build()
    nc = _CACHE["nc"]
    in_maps = _prep_inputs(
        np.asarray(x, np.float32),
        [np.asarray(w, np.float32) for w in (w0, w1, w2, w3)],
        (g0, g1, g2, g3), (be0, be1, be2, be3))
    res = run_bass_kernel_spmd(nc, in_maps, list(range(NCORES)))
    D = np.empty((8192, 8192), np.float32)
    for k in range(NCORES):
        o = np.asarray(res.results[k]["out"]).astype(np.float32)
        for c in range(2):
            D[c * 4096 + k * 512: c * 4096 + (k + 1) * 512, :] = \
                o[c * 512:(c + 1) * 512, :]
    np.fill_diagonal(D, 0.0)
    return D
